# revision 40
# baseline (speedup 1.0000x reference)
"""Causal self-attention (flipped mask: attend to k >= q) on 8 Trainium2 cores.

Sharding: 2-way data parallel over batch x 4-way head parallel (4 heads/core).
Each core computes x[b] -> qkv (its 4 heads) -> attention -> partial out-proj
(its 256 rows of Wo); the host sums the 4 partials per batch (tensor-parallel
unshard) to produce the full [B, T, C] output.

v2 changes vs v1 (317us baseline):
  - x is transposed on the HOST; the kernel loads xT [C, T] straight into
    SBUF (v1's DMA-transpose was descriptor-rate-bound: ~60us of PE idle).
  - Phase C is chunk-level software-pipelined: per (head-pair g, q-chunk n),
    ALL score matmuls are emitted as one dense PE run into paired [128,1024]
    PSUM tiles (even head in bank 0, odd head in bank 1; the two K=64
    matmuls pack concurrently into row-groups 0-1/2-3), exp consumes the
    pair in one N=1024 ACT instruction, and the AV pass trails one chunk
    behind reading e-tiles from SBUF.  This keeps the PE instruction queue
    deep (v1 gated every matmul on a just-finished ACT op -> queue
    starvation -> HAM re-throttled the PE to 1.2 GHz for all of phase C).
  - Normalization avoids v1's SBUF->SBUF DMA round-trips: reciprocal runs
    directly on the PSUM denominator rows, gpsimd partition_broadcast
    spreads 1/sum (AP base-partition aware), DVE multiplies into yT.
  - Out-projection + output DMA are interleaved per q-chunk instead of a
    serial tail.
  - V bias is folded into the projection PSUM chain via a K=1 matmul.
"""

import numpy as np

B, T, C = 2, 2048, 1024
H = 16
D = 64
NH = 4           # heads per core
HC = NH * D      # 256 local head cols
SCALE = 0.125    # 1/sqrt(D)
NEG = -1.0e5
N_CORES = 8

NT = T // 128    # 16 t-tiles
NCC = C // 128   # 8 c-chunks
NQ = T // 512    # 4 q-chunks of 512
NJ = T // 128    # 16 kt-chunks of 128

_CACHE = {}


def _build_nc():
    import concourse.tile as tile
    from concourse import bacc, mybir

    f32 = mybir.dt.float32
    f16 = mybir.dt.float16
    Exp = mybir.ActivationFunctionType.Exp
    Ident = mybir.ActivationFunctionType.Identity

    nc = bacc.Bacc(None, target_bir_lowering=False, debug=False)

    # weights/x arrive host-prearranged in SBUF layout (partition-major) so
    # every input DMA is contiguous: descriptor-light issues, and split into
    # pieces so multiple DMA queues (~25GB/s each) work in parallel
    xt = nc.dram_tensor("xt", [128, NCC, T], f16, kind="ExternalInput")
    wq = nc.dram_tensor("wq", [128, NCC, HC], f16, kind="ExternalInput")
    wk = nc.dram_tensor("wk", [128, NCC, HC], f16, kind="ExternalInput")
    wv = nc.dram_tensor("wv", [128, NCC, HC], f16, kind="ExternalInput")
    bqs = nc.dram_tensor("bqs", [HC], f32, kind="ExternalInput")
    bk = nc.dram_tensor("bk", [HC], f32, kind="ExternalInput")
    bv = nc.dram_tensor("bv", [1, HC], f16, kind="ExternalInput")
    ones1 = nc.dram_tensor("ones1", [1, 128], f16, kind="ExternalInput")
    ident = nc.dram_tensor("ident", [128, 128], f16, kind="ExternalInput")
    wo = nc.dram_tensor("wo", [128, 2, C], f16, kind="ExternalInput")
    bob = nc.dram_tensor("bob", [128, C], f16, kind="ExternalInput")
    mskB = nc.dram_tensor("mskB", [128, 896], f16, kind="ExternalInput")
    out = nc.dram_tensor("out", [T, C], f16, kind="ExternalOutput")

    with tile.TileContext(nc) as tc, (
        tc.tile_pool(name="consts", bufs=1)) as consts, (
        tc.tile_pool(name="wts", bufs=1)) as wts, (
        tc.tile_pool(name="persist", bufs=1)) as persist, (
        tc.tile_pool(name="xTp", bufs=1)) as xTp, (
        tc.tile_pool(name="epool", bufs=24)) as epool, (
        tc.tile_pool(name="rpool", bufs=2)) as rpool, (
        tc.tile_pool(name="opool", bufs=3)) as opool, (
        tc.tile_pool(name="psS", bufs=2, space="PSUM")) as psS:

        # ---- input DMAs (xT + weights first: they gate the qk chains) ----
        # issue engines work in parallel: sync takes xT (16 queues), scalar
        # takes qkv weights, gpsimd takes the late-needed consts
        xT_sb = xTp.tile([128, NCC, T], f16)
        for c0 in range(NCC):
            for h in range(2):
                nc.sync.dma_start(
                    out=xT_sb[:, c0, h * 1024:(h + 1) * 1024],
                    in_=xt[:, c0, h * 1024:(h + 1) * 1024],
                )
        wk_sb = wts.tile([128, NCC, HC], f16)
        wq_sb = wts.tile([128, NCC, HC], f16)
        wv_sb = wts.tile([128, NCC, HC], f16)
        for wsb, wdr in ((wk_sb, wk), (wq_sb, wq), (wv_sb, wv)):
            for q in range(4):
                nc.scalar.dma_start(
                    out=wsb[:, 2 * q:2 * q + 2, :], in_=wdr[:, 2 * q:2 * q + 2, :]
                )

        bq_sb = consts.tile([128, 2], f32)
        nc.gpsimd.dma_start(out=bq_sb, in_=bqs.rearrange("(a p) -> p a", p=128))
        bk_sb = consts.tile([128, 2], f32)
        nc.gpsimd.dma_start(out=bk_sb, in_=bk.rearrange("(a p) -> p a", p=128))
        bv_sb = consts.tile([1, HC], f16)
        nc.gpsimd.dma_start(out=bv_sb, in_=bv[:, :])
        ones_sb = consts.tile([1, 128], f16)
        nc.gpsimd.dma_start(out=ones_sb, in_=ones1[:, :])
        ident_sb = consts.tile([128, 128], f16)
        nc.gpsimd.dma_start(out=ident_sb, in_=ident[:, :])
        msk_sb = consts.tile([128, 896], f16)
        nc.gpsimd.dma_start(out=msk_sb, in_=mskB[:, :])
        wo_sb = wts.tile([128, 2, C], f16)
        for g in range(2):
            nc.gpsimd.dma_start(out=wo_sb[:, g, :], in_=wo[:, g, :])
        bob_sb = consts.tile([128, C], f16)
        nc.gpsimd.dma_start(out=bob_sb, in_=bob[:, :])

        # ---- persistent activations ----
        qT_sb = persist.tile([128, 2, T], f16)   # [even-head dims | odd] per pair g
        kT_sb = persist.tile([128, 2, T], f16)
        # v, augmented: per t-tile, per pair g: [65 even | 128 odd]
        # even block: cols 0..63 = v(2g), col 64 = 1.0
        # odd block:  col 65 = 1.0, cols 66..128 = 0, cols 129..192 = v(2g+1)
        v_sb = persist.tile([128, NT, 2, 195], f16)
        yT_sb = persist.tile([128, 2, T], f16)

        # ones columns + zero block, written on-chip instead of DMAing a
        # mostly-zero 1.6MB constant
        nc.vector.memset(v_sb[:, :, :, 64:66], 1.0)
        nc.vector.memset(v_sb[:, :, :, 66:129], 0.0)

        # ---- phase B emission helpers ----
        def emit_qk(psB, g):
            for is_q in range(2):
                w_sb = wq_sb if is_q else wk_sb
                dst = qT_sb if is_q else kT_sb
                bias = bq_sb if is_q else bk_sb
                scale = SCALE if is_q else 1.0
                for m in range(NQ):
                    ps = psB.tile([128, 512], f32, tag="pqk")
                    for c0 in range(NCC):
                        nc.tensor.matmul(
                            ps,
                            lhsT=w_sb[:, c0, g * 128:(g + 1) * 128],
                            rhs=xT_sb[:, c0, m * 512:(m + 1) * 512],
                            start=(c0 == 0), stop=(c0 == NCC - 1),
                        )
                    nc.scalar.activation(
                        dst[:, g, m * 512:(m + 1) * 512], ps, Ident,
                        bias=bias[:, g:g + 1], scale=scale,
                    )

        def emit_v(psB):
            for t0 in range(NT):
                psv = psB.tile([128, HC], f32, tag="pv")
                for c0 in range(NCC):
                    nc.tensor.matmul(
                        psv,
                        lhsT=xT_sb[:, c0, t0 * 128:(t0 + 1) * 128],
                        rhs=wv_sb[:, c0, :],
                        start=(c0 == 0), stop=False,
                    )
                nc.tensor.matmul(
                    psv, lhsT=ones_sb[0:1, :], rhs=bv_sb[0:1, :],
                    start=False, stop=True,
                )
                pv4 = psv.rearrange("p (g w d) -> p g w d", g=2, w=2)
                nc.vector.tensor_copy(v_sb[:, t0, :, 0:64], pv4[:, :, 0, :])
                nc.vector.tensor_copy(v_sb[:, t0, :, 129:193], pv4[:, :, 1, :])

        # ---- phase C emission helpers ----
        def emit_attn_chunk(psY, g, n):
                qs = n * 512
                # scores pass: dense run of paired K=64 matmuls
                es = []
                for j in range(4 * n, NJ):
                    ks = j * 128
                    band = j < 4 * n + 4
                    # on the diagonal band, queries beyond the block's key
                    # range are fully masked: the full-width mask matmul
                    # (-6e4 there) goes FIRST (start=True), and the score
                    # pair only computes the nj live columns on top of it
                    nj = 128 * (j - 4 * n + 1) if band else 512
                    s_t = psS.tile([128, 1024], f32, tag="s")
                    if band:
                        # additive -6e4 mask via identity matmul (PE, not DVE)
                        o = 128 * j - 512 * n
                        nc.tensor.matmul(
                            s_t[:, 0:512],
                            lhsT=ident_sb,
                            rhs=msk_sb[:, 384 - o:896 - o],
                            start=True, stop=False,
                        )
                        nc.tensor.matmul(
                            s_t[:, 512:1024],
                            lhsT=ident_sb,
                            rhs=msk_sb[:, 384 - o:896 - o],
                            start=True, stop=False,
                        )
                    nc.tensor.matmul(
                        s_t[:, 0:nj],
                        lhsT=kT_sb[0:64, g, ks:ks + 128],
                        rhs=qT_sb[0:64, g, qs:qs + nj],
                        start=not band, stop=True,
                    )
                    nc.tensor.matmul(
                        s_t[:, 512:512 + nj],
                        lhsT=kT_sb[64:128, g, ks:ks + 128],
                        rhs=qT_sb[64:128, g, qs:qs + nj],
                        start=not band, stop=True,
                    )
                    e_t = epool.tile([128, 1024], f16, tag="e")
                    nc.scalar.activation(e_t, s_t, Exp)
                    es.append(e_t)
                # AV pass: dense accumulation, e-tiles produced a pass ago
                y_t = psY.tile([128, 1024], f32, tag="y")
                for idx, j in enumerate(range(4 * n, NJ)):
                    e_t = es[idx]
                    nc.tensor.matmul(
                        y_t[0:65, 0:512],
                        lhsT=v_sb[:, j, g, 0:65],
                        rhs=e_t[:, 0:512],
                        start=(j == 4 * n), stop=(j == NJ - 1),
                    )
                    nc.tensor.matmul(
                        y_t[:, 512:1024],
                        lhsT=v_sb[:, j, g, 65:193],
                        rhs=e_t[:, 512:1024],
                        start=(j == 4 * n), stop=(j == NJ - 1),
                    )
                # normalize: denom_e on psum partition 64 (cols 0:512),
                # denom_o on partition 0 (cols 512:1024).  Reciprocal is
                # ~6.5ns/elem PER LANE, so reshape [1,512] -> [128,4] by DMA
                # first (one-lane recip would cost 3.3us); HW
                # partition_broadcast needs full-dst + src partition 0, so
                # DMA the reciprocals back to a partition-0 row.
                tmp = rpool.tile([128, 512], f32, tag="tmp")
                nc.vector.tensor_copy(tmp[64:65, :], y_t[64:65, 0:512])
                nc.vector.tensor_copy(tmp[0:1, :], y_t[0:1, 512:1024])
                rs = rpool.tile([128, 8], f32, tag="rs")
                nc.sync.dma_start(out=rs[:, 0:4], in_=tmp[64:65, :])
                nc.sync.dma_start(out=rs[:, 4:8], in_=tmp[0:1, :])
                rr = rpool.tile([128, 8], f32, tag="rr")
                nc.vector.reciprocal(rr, rs)
                rt = rpool.tile([128, 1024], f32, tag="rt")
                nc.sync.dma_start(out=rt[0:1, 0:512], in_=rr[:, 0:4])
                nc.sync.dma_start(out=rt[0:1, 512:1024], in_=rr[:, 4:8])
                bsbE = rpool.tile([128, 512], f32, tag="bsbE")
                bsbO = rpool.tile([128, 512], f32, tag="bsbO")
                nc.gpsimd.partition_broadcast(bsbE[:, :], rt[0:1, 0:512])
                nc.gpsimd.partition_broadcast(bsbO[:, :], rt[0:1, 512:1024])
                nc.vector.tensor_mul(
                    yT_sb[0:64, g, qs:qs + 512], y_t[0:64, 0:512], bsbE[0:64, :]
                )
                nc.vector.tensor_mul(
                    yT_sb[64:128, g, qs:qs + 512], y_t[64:128, 512:1024], bsbO[64:128, :]
                )

        def emit_outproj(psY, n):
            for t0 in range(4 * n, 4 * n + 4):
                pd = psS.tile([128, 1024], f32, tag="s")
                for g in range(2):
                    nc.tensor.matmul(
                        pd[:, 0:512],
                        lhsT=yT_sb[:, g, t0 * 128:(t0 + 1) * 128],
                        rhs=wo_sb[:, g, 0:512],
                        start=(g == 0), stop=(g == 1),
                    )
                    nc.tensor.matmul(
                        pd[:, 512:1024],
                        lhsT=yT_sb[:, g, t0 * 128:(t0 + 1) * 128],
                        rhs=wo_sb[:, g, 512:1024],
                        start=(g == 0), stop=(g == 1),
                    )
                o_sb = opool.tile([128, C], f16, tag="o")
                nc.vector.tensor_add(o_sb[:, 0:512], pd[:, 0:512], bob_sb[:, 0:512])
                nc.vector.tensor_add(o_sb[:, 512:1024], pd[:, 512:1024], bob_sb[:, 512:1024])
                nc.sync.dma_start(out=out[t0 * 128:(t0 + 1) * 128, :], in_=o_sb)

        # ---- emission order + priorities ----
        # PSUM budget forces psB (4 banks) to close before psY (4 banks)
        # opens, so B is emitted first; g0's attention is then emitted with
        # priorities rebased to just after qk(g0) so the scheduler prefers
        # feeding ACT (scores -> exp) and uses v/qk(g1) as stall filler.
        with tc.tile_pool(name="psB", bufs=2, space="PSUM") as psB:
            emit_qk(psB, 0)
            prio_after_qk0 = tc.cur_priority
            emit_v(psB)
            emit_qk(psB, 1)
        with tc.tile_pool(name="psY", bufs=2, space="PSUM") as psY:
            with tc.high_priority(offset=tc.cur_priority - prio_after_qk0):
                for n in range(NQ):
                    emit_attn_chunk(psY, 0, n)
            for n in range(NQ):
                emit_attn_chunk(psY, 1, n)
                emit_outproj(psY, n)

    nc.compile()
    return nc


def _host_consts():
    w = np.arange(896)[None, :]
    p = np.arange(128)[:, None]
    mskB = np.where(p >= w - 384, 0.0, -60000.0).astype(np.float16)
    ones1 = np.ones((1, 128), dtype=np.float16)
    ident = np.eye(128, dtype=np.float16)
    return mskB, ones1, ident


def make_in_maps(x, Wqkv, bqkv, Wo, bo):
    x = np.asarray(x, dtype=np.float32)
    Wqkv = np.asarray(Wqkv, dtype=np.float32)
    bqkv = np.asarray(bqkv, dtype=np.float32)
    Wo = np.asarray(Wo, dtype=np.float32)
    bo = np.asarray(bo, dtype=np.float32)
    mskB, ones1, ident = _host_consts()

    def pmajor(w):  # [(a p), n] -> [p, a, n], the SBUF layout
        a = w.shape[0] // 128
        return np.ascontiguousarray(
            w.reshape(a, 128, -1).transpose(1, 0, 2)
        ).astype(np.float16)

    in_maps = []
    for core in range(N_CORES):
        b, hg = divmod(core, 4)
        s = HC * hg
        bob = np.broadcast_to(bo, (128, C)) if hg == 0 else np.zeros((128, C), np.float32)
        in_maps.append({
            "xt": pmajor(np.ascontiguousarray(x[b].T)),
            "wq": pmajor(Wqkv[:, s:s + HC]),
            "wk": pmajor(Wqkv[:, C + s:C + s + HC]),
            "wv": pmajor(Wqkv[:, 2 * C + s:2 * C + s + HC]),
            "bqs": np.ascontiguousarray(bqkv[s:s + HC]) * np.float32(SCALE),
            "bk": np.ascontiguousarray(bqkv[C + s:C + s + HC]),
            "bv": np.ascontiguousarray(
                bqkv[2 * C + s:2 * C + s + HC].reshape(1, HC)
            ).astype(np.float16),
            "ones1": ones1,
            "ident": ident,
            "wo": pmajor(Wo[s:s + HC, :]),
            "bob": np.ascontiguousarray(bob).astype(np.float16),
            "mskB": mskB,
        })
    return in_maps


def unshard(results):
    out = np.empty((B, T, C), dtype=np.float32)
    for b in range(B):
        acc = results[4 * b]["out"].astype(np.float32)
        for hg in range(1, 4):
            acc = acc + results[4 * b + hg]["out"].astype(np.float32)
        out[b] = acc
    return out


def get_nc():
    if "nc" not in _CACHE:
        _CACHE["nc"] = _build_nc()
    return _CACHE["nc"]


def kernel(x, Wqkv, bqkv, Wo, bo):
    from concourse.bass_utils import run_bass_kernel_spmd

    nc = get_nc()
    in_maps = make_in_maps(x, Wqkv, bqkv, Wo, bo)
    res = run_bass_kernel_spmd(nc, in_maps, list(range(N_CORES)))
    return unshard(res.results)


# revision 43
# speedup vs baseline: 1.1179x; 1.1179x over previous
"""Causal self-attention (flipped mask: attend to k >= q) on 8 Trainium2 cores.

Sharding: 2-way data parallel over batch x 4-way head parallel (4 heads/core).
Each core computes x[b] -> qkv (its 4 heads) -> attention -> partial out-proj
(its 256 rows of Wo); the host sums the 4 partials per batch (tensor-parallel
unshard) to produce the full [B, T, C] output.

v2 changes vs v1 (317us baseline):
  - x is transposed on the HOST; the kernel loads xT [C, T] straight into
    SBUF (v1's DMA-transpose was descriptor-rate-bound: ~60us of PE idle).
  - Phase C is chunk-level software-pipelined: per (head-pair g, q-chunk n),
    ALL score matmuls are emitted as one dense PE run into paired [128,1024]
    PSUM tiles (even head in bank 0, odd head in bank 1; the two K=64
    matmuls pack concurrently into row-groups 0-1/2-3), exp consumes the
    pair in one N=1024 ACT instruction, and the AV pass trails one chunk
    behind reading e-tiles from SBUF.  This keeps the PE instruction queue
    deep (v1 gated every matmul on a just-finished ACT op -> queue
    starvation -> HAM re-throttled the PE to 1.2 GHz for all of phase C).
  - Normalization avoids v1's SBUF->SBUF DMA round-trips: reciprocal runs
    directly on the PSUM denominator rows, gpsimd partition_broadcast
    spreads 1/sum (AP base-partition aware), DVE multiplies into yT.
  - Out-projection + output DMA are interleaved per q-chunk instead of a
    serial tail.
  - V bias is folded into the projection PSUM chain via a K=1 matmul.
"""

import numpy as np

B, T, C = 2, 2048, 1024
H = 16
D = 64
NH = 4           # heads per core
HC = NH * D      # 256 local head cols
SCALE = 0.125    # 1/sqrt(D)
NEG = -1.0e5
N_CORES = 8

NT = T // 128    # 16 t-tiles
NCC = C // 128   # 8 c-chunks
NQ = T // 512    # 4 q-chunks of 512
NJ = T // 128    # 16 kt-chunks of 128

_CACHE = {}


def _build_nc():
    import concourse.tile as tile
    from concourse import bacc, mybir

    f32 = mybir.dt.float32
    f16 = mybir.dt.float16
    Exp = mybir.ActivationFunctionType.Exp
    Ident = mybir.ActivationFunctionType.Identity

    nc = bacc.Bacc(None, target_bir_lowering=False, debug=False)

    # weights/x arrive host-prearranged in SBUF layout (partition-major) so
    # every input DMA is contiguous: descriptor-light issues, and split into
    # pieces so multiple DMA queues (~25GB/s each) work in parallel
    xt = nc.dram_tensor("xt", [128, NCC, T], f16, kind="ExternalInput")
    wq = nc.dram_tensor("wq", [128, NCC, HC], f16, kind="ExternalInput")
    wk = nc.dram_tensor("wk", [128, NCC, HC], f16, kind="ExternalInput")
    wv = nc.dram_tensor("wv", [128, NCC, HC], f16, kind="ExternalInput")
    bqs = nc.dram_tensor("bqs", [HC], f32, kind="ExternalInput")
    bk = nc.dram_tensor("bk", [HC], f32, kind="ExternalInput")
    bv = nc.dram_tensor("bv", [1, HC], f16, kind="ExternalInput")
    ones1 = nc.dram_tensor("ones1", [1, 128], f16, kind="ExternalInput")
    ident = nc.dram_tensor("ident", [128, 128], f16, kind="ExternalInput")
    wo = nc.dram_tensor("wo", [128, 2, C], f16, kind="ExternalInput")
    bob = nc.dram_tensor("bob", [128, C], f16, kind="ExternalInput")
    mskB = nc.dram_tensor("mskB", [128, 896], f16, kind="ExternalInput")
    out = nc.dram_tensor("out", [T, C], f16, kind="ExternalOutput")

    with tile.TileContext(nc) as tc, (
        tc.tile_pool(name="consts", bufs=1)) as consts, (
        tc.tile_pool(name="wts", bufs=1)) as wts, (
        tc.tile_pool(name="persist", bufs=1)) as persist, (
        tc.tile_pool(name="xTp", bufs=1)) as xTp, (
        tc.tile_pool(name="epool", bufs=24)) as epool, (
        tc.tile_pool(name="rpool", bufs=2)) as rpool, (
        tc.tile_pool(name="opool", bufs=3)) as opool, (
        tc.tile_pool(name="psS", bufs=2, space="PSUM")) as psS:

        # ---- input DMAs (xT + weights first: they gate the qk chains) ----
        # issue engines work in parallel: sync takes xT (16 queues), scalar
        # takes qkv weights, gpsimd takes the late-needed consts
        xT_sb = xTp.tile([128, NCC, T], f16)
        for c0 in range(NCC):
            for h in range(2):
                nc.sync.dma_start(
                    out=xT_sb[:, c0, h * 1024:(h + 1) * 1024],
                    in_=xt[:, c0, h * 1024:(h + 1) * 1024],
                )
        wk_sb = wts.tile([128, NCC, HC], f16)
        wq_sb = wts.tile([128, NCC, HC], f16)
        wv_sb = wts.tile([128, NCC, HC], f16)
        for wsb, wdr in ((wk_sb, wk), (wq_sb, wq), (wv_sb, wv)):
            for q in range(4):
                nc.scalar.dma_start(
                    out=wsb[:, 2 * q:2 * q + 2, :], in_=wdr[:, 2 * q:2 * q + 2, :]
                )

        bq_sb = consts.tile([128, 2], f32)
        nc.gpsimd.dma_start(out=bq_sb, in_=bqs.rearrange("(a p) -> p a", p=128))
        bk_sb = consts.tile([128, 2], f32)
        nc.gpsimd.dma_start(out=bk_sb, in_=bk.rearrange("(a p) -> p a", p=128))
        bv_sb = consts.tile([1, HC], f16)
        nc.gpsimd.dma_start(out=bv_sb, in_=bv[:, :])
        ones_sb = consts.tile([1, 128], f16)
        nc.gpsimd.dma_start(out=ones_sb, in_=ones1[:, :])
        ident_sb = consts.tile([128, 128], f16)
        nc.gpsimd.dma_start(out=ident_sb, in_=ident[:, :])
        msk_sb = consts.tile([128, 896], f16)
        nc.gpsimd.dma_start(out=msk_sb, in_=mskB[:, :])
        wo_sb = wts.tile([128, 2, C], f16)
        for g in range(2):
            nc.gpsimd.dma_start(out=wo_sb[:, g, :], in_=wo[:, g, :])
        bob_sb = consts.tile([128, C], f16)
        nc.gpsimd.dma_start(out=bob_sb, in_=bob[:, :])

        # ---- persistent activations ----
        qT_sb = persist.tile([128, 2, T], f16)   # [even-head dims | odd] per pair g
        kT_sb = persist.tile([128, 2, T], f16)
        # v, augmented: per t-tile, per pair g: [65 even | 128 odd]
        # even block: cols 0..63 = v(2g), col 64 = 1.0
        # odd block:  col 65 = 1.0, cols 66..128 = 0, cols 129..192 = v(2g+1)
        v_sb = persist.tile([128, NT, 2, 195], f16)
        yT_sb = persist.tile([128, 2, T], f16)

        # ones columns + zero block, written on-chip instead of DMAing a
        # mostly-zero 1.6MB constant
        nc.vector.memset(v_sb[:, :, :, 64:66], 1.0)
        nc.vector.memset(v_sb[:, :, :, 66:129], 0.0)

        # ---- phase B emission helpers ----
        def emit_qk(psB, g):
            for is_q in range(2):
                w_sb = wq_sb if is_q else wk_sb
                dst = qT_sb if is_q else kT_sb
                bias = bq_sb if is_q else bk_sb
                scale = SCALE if is_q else 1.0
                for m in range(NQ):
                    ps = psB.tile([128, 512], f32, tag="pqk")
                    for c0 in range(NCC):
                        nc.tensor.matmul(
                            ps,
                            lhsT=w_sb[:, c0, g * 128:(g + 1) * 128],
                            rhs=xT_sb[:, c0, m * 512:(m + 1) * 512],
                            start=(c0 == 0), stop=(c0 == NCC - 1),
                        )
                    nc.scalar.activation(
                        dst[:, g, m * 512:(m + 1) * 512], ps, Ident,
                        bias=bias[:, g:g + 1], scale=scale,
                    )

        def emit_v(psB):
            for t0 in range(NT):
                psv = psB.tile([128, HC], f32, tag="pv")
                for c0 in range(NCC):
                    nc.tensor.matmul(
                        psv,
                        lhsT=xT_sb[:, c0, t0 * 128:(t0 + 1) * 128],
                        rhs=wv_sb[:, c0, :],
                        start=(c0 == 0), stop=False,
                    )
                nc.tensor.matmul(
                    psv, lhsT=ones_sb[0:1, :], rhs=bv_sb[0:1, :],
                    start=False, stop=True,
                )
                pv4 = psv.rearrange("p (g w d) -> p g w d", g=2, w=2)
                nc.vector.tensor_copy(v_sb[:, t0, :, 0:64], pv4[:, :, 0, :])
                nc.vector.tensor_copy(v_sb[:, t0, :, 129:193], pv4[:, :, 1, :])

        # ---- phase C emission helpers ----
        def emit_attn_chunk(psY, g, n):
                qs = n * 512
                # scores pass: dense run of paired K=64 matmuls
                es = []
                for j in range(4 * n, NJ):
                    ks = j * 128
                    band = j < 4 * n + 4
                    # on the diagonal band, queries beyond the block's key
                    # range are fully masked: the full-width mask matmul
                    # (-6e4 there) goes FIRST (start=True), and the score
                    # pair only computes the nj live columns on top of it
                    nj = 128 * (j - 4 * n + 1) if band else 512
                    s_t = psS.tile([128, 1024], f32, tag="s")
                    if band:
                        # additive -6e4 mask via identity matmul (PE, not DVE)
                        o = 128 * j - 512 * n
                        nc.tensor.matmul(
                            s_t[:, 0:512],
                            lhsT=ident_sb,
                            rhs=msk_sb[:, 384 - o:896 - o],
                            start=True, stop=False,
                        )
                        nc.tensor.matmul(
                            s_t[:, 512:1024],
                            lhsT=ident_sb,
                            rhs=msk_sb[:, 384 - o:896 - o],
                            start=True, stop=False,
                        )
                    nc.tensor.matmul(
                        s_t[:, 0:nj],
                        lhsT=kT_sb[0:64, g, ks:ks + 128],
                        rhs=qT_sb[0:64, g, qs:qs + nj],
                        start=not band, stop=True,
                    )
                    nc.tensor.matmul(
                        s_t[:, 512:512 + nj],
                        lhsT=kT_sb[64:128, g, ks:ks + 128],
                        rhs=qT_sb[64:128, g, qs:qs + nj],
                        start=not band, stop=True,
                    )
                    e_t = epool.tile([128, 1024], f16, tag="e")
                    nc.scalar.activation(e_t, s_t, Exp)
                    es.append(e_t)
                # AV pass: dense accumulation, e-tiles produced a pass ago
                y_t = psY.tile([128, 1024], f32, tag="y")
                for idx, j in enumerate(range(4 * n, NJ)):
                    e_t = es[idx]
                    nc.tensor.matmul(
                        y_t[0:65, 0:512],
                        lhsT=v_sb[:, j, g, 0:65],
                        rhs=e_t[:, 0:512],
                        start=(j == 4 * n), stop=(j == NJ - 1),
                    )
                    nc.tensor.matmul(
                        y_t[:, 512:1024],
                        lhsT=v_sb[:, j, g, 65:193],
                        rhs=e_t[:, 512:1024],
                        start=(j == 4 * n), stop=(j == NJ - 1),
                    )
                # normalize: denom_e on psum partition 64 (cols 0:512),
                # denom_o on partition 0 (cols 512:1024).  Reciprocal is
                # ~6.5ns/elem PER LANE, so reshape [1,512] -> [128,4] by DMA
                # first (one-lane recip would cost 3.3us); HW
                # partition_broadcast needs full-dst + src partition 0, so
                # DMA the reciprocals back to a partition-0 row.
                tmp = rpool.tile([128, 512], f32, tag="tmp")
                nc.vector.tensor_copy(tmp[64:65, :], y_t[64:65, 0:512])
                nc.vector.tensor_copy(tmp[0:1, :], y_t[0:1, 512:1024])
                rs = rpool.tile([128, 8], f32, tag="rs")
                nc.sync.dma_start(out=rs[:, 0:4], in_=tmp[64:65, :])
                nc.sync.dma_start(out=rs[:, 4:8], in_=tmp[0:1, :])
                rr = rpool.tile([128, 8], f32, tag="rr")
                nc.vector.reciprocal(rr, rs)
                rt = rpool.tile([128, 1024], f32, tag="rt")
                nc.sync.dma_start(out=rt[0:1, 0:512], in_=rr[:, 0:4])
                nc.sync.dma_start(out=rt[0:1, 512:1024], in_=rr[:, 4:8])
                bsbE = rpool.tile([128, 512], f32, tag="bsbE")
                bsbO = rpool.tile([128, 512], f32, tag="bsbO")
                nc.gpsimd.partition_broadcast(bsbE[:, :], rt[0:1, 0:512])
                nc.gpsimd.partition_broadcast(bsbO[:, :], rt[0:1, 512:1024])
                nc.vector.tensor_mul(
                    yT_sb[0:64, g, qs:qs + 512], y_t[0:64, 0:512], bsbE[0:64, :]
                )
                nc.vector.tensor_mul(
                    yT_sb[64:128, g, qs:qs + 512], y_t[64:128, 512:1024], bsbO[64:128, :]
                )

        def emit_outproj(psY, n):
            for t0 in range(4 * n, 4 * n + 4):
                pd = psY.tile([128, 1024], f32, tag="y")
                for g in range(2):
                    nc.tensor.matmul(
                        pd[:, 0:512],
                        lhsT=yT_sb[:, g, t0 * 128:(t0 + 1) * 128],
                        rhs=wo_sb[:, g, 0:512],
                        start=(g == 0), stop=(g == 1),
                    )
                    nc.tensor.matmul(
                        pd[:, 512:1024],
                        lhsT=yT_sb[:, g, t0 * 128:(t0 + 1) * 128],
                        rhs=wo_sb[:, g, 512:1024],
                        start=(g == 0), stop=(g == 1),
                    )
                o_sb = opool.tile([128, C], f16, tag="o")
                nc.vector.tensor_add(o_sb[:, 0:512], pd[:, 0:512], bob_sb[:, 0:512])
                nc.vector.tensor_add(o_sb[:, 512:1024], pd[:, 512:1024], bob_sb[:, 512:1024])
                nc.sync.dma_start(out=out[t0 * 128:(t0 + 1) * 128, :], in_=o_sb)

        # ---- emission order + priorities ----
        # PSUM budget forces psB (4 banks) to close before psY (4 banks)
        # opens, so B is emitted first; the attention chunks are then
        # emitted n-outer (g0 and g1 interleaved, out-proj right after each
        # n) with priorities rebased to just after qk(g1), so the scheduler
        # prefers feeding ACT (scores -> exp) and uses v as stall filler,
        # and every normalize/out-proj chain has a following chunk's
        # scores/AV as PE fill.
        with tc.tile_pool(name="psB", bufs=2, space="PSUM") as psB:
            emit_qk(psB, 0)
            prio_after_qk = tc.cur_priority
            emit_qk(psB, 1)
            emit_v(psB)
        with tc.tile_pool(name="psY", bufs=2, space="PSUM") as psY:
            with tc.high_priority(offset=tc.cur_priority - prio_after_qk):
                for n in range(NQ):
                    emit_attn_chunk(psY, 0, n)
                    emit_attn_chunk(psY, 1, n)
                    emit_outproj(psY, n)

    nc.compile()
    return nc


def _host_consts():
    w = np.arange(896)[None, :]
    p = np.arange(128)[:, None]
    mskB = np.where(p >= w - 384, 0.0, -60000.0).astype(np.float16)
    ones1 = np.ones((1, 128), dtype=np.float16)
    ident = np.eye(128, dtype=np.float16)
    return mskB, ones1, ident


def make_in_maps(x, Wqkv, bqkv, Wo, bo):
    x = np.asarray(x, dtype=np.float32)
    Wqkv = np.asarray(Wqkv, dtype=np.float32)
    bqkv = np.asarray(bqkv, dtype=np.float32)
    Wo = np.asarray(Wo, dtype=np.float32)
    bo = np.asarray(bo, dtype=np.float32)
    mskB, ones1, ident = _host_consts()

    def pmajor(w):  # [(a p), n] -> [p, a, n], the SBUF layout
        a = w.shape[0] // 128
        return np.ascontiguousarray(
            w.reshape(a, 128, -1).transpose(1, 0, 2)
        ).astype(np.float16)

    in_maps = []
    for core in range(N_CORES):
        b, hg = divmod(core, 4)
        s = HC * hg
        bob = np.broadcast_to(bo, (128, C)) if hg == 0 else np.zeros((128, C), np.float32)
        in_maps.append({
            "xt": pmajor(np.ascontiguousarray(x[b].T)),
            "wq": pmajor(Wqkv[:, s:s + HC]),
            "wk": pmajor(Wqkv[:, C + s:C + s + HC]),
            "wv": pmajor(Wqkv[:, 2 * C + s:2 * C + s + HC]),
            "bqs": np.ascontiguousarray(bqkv[s:s + HC]) * np.float32(SCALE),
            "bk": np.ascontiguousarray(bqkv[C + s:C + s + HC]),
            "bv": np.ascontiguousarray(
                bqkv[2 * C + s:2 * C + s + HC].reshape(1, HC)
            ).astype(np.float16),
            "ones1": ones1,
            "ident": ident,
            "wo": pmajor(Wo[s:s + HC, :]),
            "bob": np.ascontiguousarray(bob).astype(np.float16),
            "mskB": mskB,
        })
    return in_maps


def unshard(results):
    out = np.empty((B, T, C), dtype=np.float32)
    for b in range(B):
        acc = results[4 * b]["out"].astype(np.float32)
        for hg in range(1, 4):
            acc = acc + results[4 * b + hg]["out"].astype(np.float32)
        out[b] = acc
    return out


def get_nc():
    if "nc" not in _CACHE:
        _CACHE["nc"] = _build_nc()
    return _CACHE["nc"]


def kernel(x, Wqkv, bqkv, Wo, bo):
    from concourse.bass_utils import run_bass_kernel_spmd

    nc = get_nc()
    in_maps = make_in_maps(x, Wqkv, bqkv, Wo, bo)
    res = run_bass_kernel_spmd(nc, in_maps, list(range(N_CORES)))
    return unshard(res.results)
